# revision 6
# baseline (speedup 1.0000x reference)
"""TRN2 Bass kernel for channel-attention (dense_transformer, B=8, C=512, T=4096).

Math (per batch element, C=512, T=4096):
    q = Wq x + bq; k = Wk x + bk; v = Wv x + bv          (1x1 convs)
    dots = (q k^T) * SCALE;  attn = softmax(dots, -1);  out = attn v

Reformulation (Gram trick) — minimizes T-sized contractions:
    dots*SCALE = Wq' G~ Wk'^T  with  G~ = [x;1][x;1]^T   (one big matmul)
    out = (attn [Wv|bv]) [x;1]                            (v never materialized)

Precision: softmax is sharp (logits up to ~140), so the logit path uses
split-bf16 (hi+lo) arithmetic giving ~fp32 logits; the post-softmax path is
plain bf16.  G ~= xh xh^T + S + S^T with S = xh xl^T (ll term dropped).

Sharding: data-parallel over batch — one batch element per NeuronCore.
"""
import sys
import numpy as np

for _p in ("/opt/trn_rl_repo", "/root/.axon_site/_ro/trn_rl_repo"):
    if _p not in sys.path:
        sys.path.insert(0, _p)

import ml_dtypes
import concourse.bass as bass
import concourse.tile as tile
from concourse import bacc, mybir
from concourse.bass_utils import run_bass_kernel_spmd
from concourse.masks import make_identity

F32 = mybir.dt.float32
BF16 = mybir.dt.bfloat16
AF = mybir.ActivationFunctionType
AX = mybir.AxisListType

C = 512
T = 4096
NCH = C // 128   # 4 partition chunks of the channel dim
NTT = T // 128   # 32 t-tiles (transposed layout)
NTS = T // 512   # 8 t-slices (free-dim tiles)
SCALE = np.float32(64 ** -0.5)

_NC_CACHE = []
_last_in_maps = None


def _emit(nc, tc, ctx, d):
    cs = lambda m: slice(128 * m, 128 * (m + 1))

    persist = ctx.enter_context(tc.tile_pool(name="persist", bufs=1))
    work = ctx.enter_context(tc.tile_pool(name="work", bufs=1))
    outp = ctx.enter_context(tc.tile_pool(name="outp", bufs=4))
    psum = ctx.enter_context(tc.tile_pool(name="psum", bufs=8, space="PSUM"))

    # ---- transposed loads straight from DRAM via X-bar, in T-quarters so
    # the G~ matmul stream can start as soon as the first quarter lands.
    # xT*[p, i, c] holds x^T rows in an arbitrary-but-consistent (p,i)<->t
    # mapping; the Gram contraction is t-permutation invariant.
    xTh = persist.tile([128, NTT, C], BF16, name="xTh", tag="xTh")
    xTl = persist.tile([128, NTT, C], BF16, name="xTl", tag="xTl")
    TQ = 8
    qt, qtiles = T // TQ, NTT // TQ
    for q in range(TQ):
        nc.sync.dma_start(xTh[:, q * qtiles:(q + 1) * qtiles, :],
                          d["xh"][:, q * qt:(q + 1) * qt], transpose=True)
    for q in range(TQ):
        nc.sync.dma_start(xTl[:, q * qtiles:(q + 1) * qtiles, :],
                          d["xl"][:, q * qt:(q + 1) * qt], transpose=True)

    # constants (gpsimd/DVE; ready before PE needs them)
    ident = persist.tile([128, 128], BF16, name="ident", tag="ident")
    make_identity(nc, ident[:])
    one_1 = persist.tile([1, 1], BF16, name="one_1", tag="one_1")
    nc.vector.memset(one_1[:], 1.0)
    t_const = persist.tile([1, 1], BF16, name="tconst", tag="tconst")
    nc.vector.memset(t_const[:], float(T))

    # ---- G~ part 1: Ghh = xh xh^T, i-major so PE chases the transposes ----
    psG = [psum.tile([128, C], F32, name="mm", tag="mm") for _ in range(NCH)]
    for i in range(NTT):
        for m in range(NCH):
            nc.tensor.matmul(psG[m][:], xTh[:, i, cs(m)], xTh[:, i, :],
                             start=(i == 0), stop=False)

    # ---- normal-layout x + weights (issued after the transposes) ----
    x_bf = [persist.tile([128, T], BF16, name=f"xbf{c}", tag=f"xbf{c}")
            for c in range(NCH)]
    for c in range(NCH):
        nc.sync.dma_start(x_bf[c][:], d["xh"][cs(c), :])

    def wchunks(name):
        tiles = []
        for k in range(NCH):
            t_ = persist.tile([128, C], BF16, name=f"{name}{k}", tag=f"{name}{k}")
            nc.sync.dma_start(t_[:], d[name][cs(k), :])
            tiles.append(t_)
        return tiles

    wkt_h, wkt_l = wchunks("wkt_h"), wchunks("wkt_l")
    wqt_h, wqt_l = wchunks("wqt_h"), wchunks("wqt_l")
    wv = wchunks("wv")
    bv = []
    for k in range(NCH):
        t_ = persist.tile([128, 1], BF16, name=f"bv{k}", tag=f"bv{k}")
        nc.sync.dma_start(t_[:], d["bv"][cs(k), :])
        bv.append(t_)
    rows = {}
    for name in ("bq_h", "bq_l", "bk_h", "bk_l"):
        t_ = persist.tile([1, C], BF16, name=name, tag=name)
        nc.sync.dma_start(t_[:], d[name][:])
        rows[name] = t_

    # ---- G~ part 2: S = xh xl^T ----
    psS = [psum.tile([128, C], F32, name="mm", tag="mm") for _ in range(NCH)]
    for i in range(NTT):
        for m in range(NCH):
            nc.tensor.matmul(psS[m][:], xTh[:, i, cs(m)], xTl[:, i, :],
                             start=(i == 0), stop=(i == NTT - 1))
    S_sb = []
    for m in range(NCH):
        s_sb = work.tile([128, C], BF16, name=f"S{m}", tag=f"S{m}")
        nc.scalar.copy(s_sb[:], psS[m][:])
        S_sb.append(s_sb)

    # ---- xs = row sums of x (hi part; lo contribution negligible) ----
    xs_h, xs_l = [], []
    for c in range(NCH):
        xsf = work.tile([128, 1], F32, name=f"xsf{c}", tag=f"xsf{c}")
        nc.vector.reduce_sum(xsf[:], x_bf[c][:], axis=AX.X)
        h = work.tile([128, 1], BF16, name=f"xsh{c}", tag=f"xsh{c}")
        l = work.tile([128, 1], BF16, name=f"xsl{c}", tag=f"xsl{c}")
        nc.vector.tensor_copy(h[:], xsf[:])
        nc.vector.tensor_sub(l[:], xsf[:], h[:])
        xs_h.append(h)
        xs_l.append(l)

    # ---- G~ assembly: psG += S + S^T; drain to split-bf16 ----
    Gh, Gl = [], []
    for m in range(NCH):
        nc.tensor.matmul(psG[m][:], ident[:], S_sb[m][:], start=False, stop=False)
        for j in range(NCH):
            nc.tensor.matmul(psG[m][:, cs(j)], S_sb[j][:, cs(m)], ident[:],
                             start=False, stop=(j == NCH - 1))
        gh = work.tile([128, C], BF16, name=f"Gh{m}", tag=f"Gh{m}")
        gl = work.tile([128, C], BF16, name=f"Gl{m}", tag=f"Gl{m}")
        nc.scalar.copy(gh[:], psG[m][:])
        nc.vector.tensor_sub(gl[:], psG[m][:], gh[:])
        Gh.append(gh)
        Gl.append(gl)

    # xs^T rows [1, C] via transpose-matmul (after S psums freed)
    ps_h = psum.tile([1, C], F32, name="mm", tag="mm")
    ps_l = psum.tile([1, C], F32, name="mm", tag="mm")
    for c in range(NCH):
        nc.tensor.matmul(ps_h[:, cs(c)], xs_h[c][:], ident[:], start=True, stop=True)
        nc.tensor.matmul(ps_l[:, cs(c)], xs_l[c][:], ident[:], start=True, stop=True)
    xsT_h = work.tile([1, C], BF16, name="xsTh", tag="xsTh")
    xsT_l = work.tile([1, C], BF16, name="xsTl", tag="xsTl")
    nc.scalar.copy(xsT_h[:], ps_h[:])
    nc.scalar.copy(xsT_l[:], ps_l[:])

    # ---- Z = G~ Wk'^T  (main chunks split-bf16 + rank-1 bias fringe) ----
    Zh, Zl = [], []
    for m in range(NCH):
        ps = psum.tile([128, C], F32, name="mm", tag="mm")
        first = True
        for k in range(NCH):
            for lh, rh in ((Gh[k], wkt_h[k]), (Gh[k], wkt_l[k]), (Gl[k], wkt_h[k])):
                nc.tensor.matmul(ps[:], lh[:, cs(m)], rh[:], start=first, stop=False)
                first = False
        nc.tensor.matmul(ps[:], xsT_h[:, cs(m)], rows["bk_h"][:], start=False, stop=False)
        nc.tensor.matmul(ps[:], xsT_h[:, cs(m)], rows["bk_l"][:], start=False, stop=False)
        nc.tensor.matmul(ps[:], xsT_l[:, cs(m)], rows["bk_h"][:], start=False, stop=True)
        zh = work.tile([128, C], BF16, name=f"Zh{m}", tag=f"Zh{m}")
        zl = work.tile([128, C], BF16, name=f"Zl{m}", tag=f"Zl{m}")
        nc.scalar.copy(zh[:], ps[:])
        nc.vector.tensor_sub(zl[:], ps[:], zh[:])
        Zh.append(zh)
        Zl.append(zl)
    # fringe row of Z:  Zr = xs^T Wk^T + T*bk
    ps = psum.tile([1, C], F32, name="mm", tag="mm")
    first = True
    for k in range(NCH):
        for lh, rh in ((xs_h[k], wkt_h[k]), (xs_h[k], wkt_l[k]), (xs_l[k], wkt_h[k])):
            nc.tensor.matmul(ps[:], lh[:], rh[:], start=first, stop=False)
            first = False
    nc.tensor.matmul(ps[:], t_const[:], rows["bk_h"][:], start=False, stop=False)
    nc.tensor.matmul(ps[:], t_const[:], rows["bk_l"][:], start=False, stop=True)
    Zr_h = work.tile([1, C], BF16, name="Zrh", tag="Zrh")
    Zr_l = work.tile([1, C], BF16, name="Zrl", tag="Zrl")
    nc.scalar.copy(Zr_h[:], ps[:])
    nc.vector.tensor_sub(Zr_l[:], ps[:], Zr_h[:])

    # ---- dots = Wq' Z~ ; fused softmax -> attn_un (bf16) + diag(1/sum) ----
    attn_un, diag = [], []
    for m in range(NCH):
        ps = psum.tile([128, C], F32, name="mm", tag="mm")
        first = True
        for k in range(NCH):
            for lh, rh in ((wqt_h[k], Zh[k]), (wqt_h[k], Zl[k]), (wqt_l[k], Zh[k])):
                nc.tensor.matmul(ps[:], lh[:, cs(m)], rh[:], start=first, stop=False)
                first = False
        nc.tensor.matmul(ps[:], rows["bq_h"][:, cs(m)], Zr_h[:], start=False, stop=False)
        nc.tensor.matmul(ps[:], rows["bq_h"][:, cs(m)], Zr_l[:], start=False, stop=False)
        nc.tensor.matmul(ps[:], rows["bq_l"][:, cs(m)], Zr_h[:], start=False, stop=True)
        nmx = work.tile([128, 1], F32, name=f"nmx{m}", tag=f"nmx{m}")
        nc.vector.reduce_max(nmx[:], ps[:], axis=AX.X, negate=True)
        au = work.tile([128, C], BF16, name=f"au{m}", tag=f"au{m}")
        sm = work.tile([128, 1], F32, name=f"sm{m}", tag=f"sm{m}")
        nc.vector.memset(sm[:], 0.0)
        nc.scalar.activation(au[:], ps[:], AF.Exp, bias=nmx[:], scale=1.0,
                             accum_out=sm[:])
        ri = work.tile([128, 1], F32, name=f"ri{m}", tag=f"ri{m}")
        nc.vector.reciprocal(ri[:], sm[:])
        dg = work.tile([128, 128], BF16, name=f"dg{m}", tag=f"dg{m}")
        nc.vector.tensor_scalar_mul(dg[:], ident[:], ri[:])
        attn_un.append(au)
        diag.append(dg)

    # ---- attn^T (normalized) via matmul with diag(1/sum) rhs ----
    attnT = []
    for j in range(NCH):
        ps = psum.tile([128, C], F32, name="mm", tag="mm")
        for m in range(NCH):
            nc.tensor.matmul(ps[:, cs(m)], attn_un[m][:, cs(j)], diag[m][:],
                             start=True, stop=True)
        at = work.tile([128, C], BF16, name=f"at{j}", tag=f"at{j}")
        nc.scalar.copy(at[:], ps[:])
        attnT.append(at)

    # ---- P~^T = [Wv|bv]^T attn^T ----
    Pt = []
    for jm in range(NCH):
        ps = psum.tile([128, C], F32, name="mm", tag="mm")
        for k in range(NCH):
            nc.tensor.matmul(ps[:], wv[k][:, cs(jm)], attnT[k][:],
                             start=(k == 0), stop=(k == NCH - 1))
        pt = work.tile([128, C], BF16, name=f"pt{jm}", tag=f"pt{jm}")
        nc.scalar.copy(pt[:], ps[:])
        Pt.append(pt)
    # r = attn bv  (as a [1, C] row), then transposed to per-chunk [128, 1]
    ps = psum.tile([1, C], F32, name="mm", tag="mm")
    for k in range(NCH):
        nc.tensor.matmul(ps[:], bv[k][:], attnT[k][:],
                         start=(k == 0), stop=(k == NCH - 1))
    r_b = work.tile([1, C], BF16, name="rb", tag="rb")
    nc.scalar.copy(r_b[:], ps[:])
    rT = []
    ps_rt = psum.tile([128, NCH], F32, name="mm", tag="mm")
    for m in range(NCH):
        nc.tensor.matmul(ps_rt[:, m:m + 1], r_b[:, cs(m)], one_1[:],
                         start=True, stop=True)
    for m in range(NCH):
        rt = work.tile([128, 1], F32, name=f"rT{m}", tag=f"rT{m}")
        nc.vector.tensor_copy(rt[:], ps_rt[:, m:m + 1])
        rT.append(rt)

    # ---- out = P x + r  (bias folded into the activation drain) ----
    for m in range(NCH):
        for ts in range(NTS):
            sl = slice(512 * ts, 512 * (ts + 1))
            ps = psum.tile([128, 512], F32, name="mm", tag="mm")
            for k in range(NCH):
                nc.tensor.matmul(ps[:], Pt[k][:, cs(m)], x_bf[k][:, sl],
                                 start=(k == 0), stop=(k == NCH - 1))
            ob = outp.tile([128, 512], F32, name="ob", tag="ob")
            nc.scalar.activation(ob[:], ps[:], AF.Identity, bias=rT[m][:],
                                 scale=1.0)
            nc.sync.dma_start(d["out"][cs(m), sl], ob[:])


def _build_nc():
    from contextlib import ExitStack
    nc = bacc.Bacc()
    d = {}
    d["xh"] = nc.declare_dram_parameter("xh", [C, T], BF16, isOutput=False)
    d["xl"] = nc.declare_dram_parameter("xl", [C, T], BF16, isOutput=False)
    for name in ("wqt_h", "wqt_l", "wkt_h", "wkt_l", "wv"):
        d[name] = nc.declare_dram_parameter(name, [C, C], BF16, isOutput=False)
    for name in ("bq_h", "bq_l", "bk_h", "bk_l"):
        d[name] = nc.declare_dram_parameter(name, [1, C], BF16, isOutput=False)
    d["bv"] = nc.declare_dram_parameter("bv", [C, 1], BF16, isOutput=False)
    d["out"] = nc.declare_dram_parameter("out", [C, T], F32, isOutput=True)

    with tile.TileContext(nc) as tc:
        with ExitStack() as ctx:
            _emit(nc, tc, ctx, d)
    nc.finalize()
    return nc


def _split_bf16(a):
    h = a.astype(ml_dtypes.bfloat16)
    l = (a.astype(np.float32) - h.astype(np.float32)).astype(ml_dtypes.bfloat16)
    return h, l


def kernel(x, Wq, bq, Wk, bk, Wv, bv):
    x = np.ascontiguousarray(np.asarray(x, dtype=np.float32))
    B = x.shape[0]
    assert x.shape == (B, C, T)

    wqt = np.ascontiguousarray(Wq.T.astype(np.float32) * SCALE)
    wkt = np.ascontiguousarray(Wk.T.astype(np.float32))
    wqt_h, wqt_l = _split_bf16(wqt)
    wkt_h, wkt_l = _split_bf16(wkt)
    bq_h, bq_l = _split_bf16((bq.astype(np.float32) * SCALE)[None, :])
    bk_h, bk_l = _split_bf16(bk.astype(np.float32)[None, :])
    wv_b = Wv.astype(np.float32).astype(ml_dtypes.bfloat16)
    bv_b = bv.astype(np.float32)[:, None].astype(ml_dtypes.bfloat16)

    shared = dict(wqt_h=wqt_h, wqt_l=wqt_l, wkt_h=wkt_h, wkt_l=wkt_l,
                  bq_h=bq_h, bq_l=bq_l, bk_h=bk_h, bk_l=bk_l,
                  wv=wv_b, bv=bv_b)

    in_maps = []
    for b in range(B):
        xh, xl = _split_bf16(x[b])
        in_maps.append(dict(shared, xh=np.ascontiguousarray(xh),
                            xl=np.ascontiguousarray(xl)))

    if not _NC_CACHE:
        _NC_CACHE.append(_build_nc())
    nc = _NC_CACHE[0]

    global _last_in_maps
    _last_in_maps = in_maps

    res = run_bass_kernel_spmd(nc, in_maps, list(range(B)))
    return np.stack([res.results[b]["out"] for b in range(B)], axis=0)


# revision 11
# speedup vs baseline: 16.5640x; 16.5640x over previous
"""TRN2 Bass kernel for channel-attention (dense_transformer, B=8, C=512, T=4096).

Math (per batch element, C=512, T=4096):
    q = Wq x + bq; k = Wk x + bk; v = Wv x + bv          (1x1 convs)
    dots = (q k^T) * SCALE;  attn = softmax(dots, -1);  out = attn v

Reformulation (Gram trick) — minimizes T-sized contractions:
    dots*SCALE = Wq' G~ Wk'^T  with  G~ = [x;1][x;1]^T   (one big matmul)
    out = (attn [Wv|bv]) [x;1]                            (v never materialized)

Precision: softmax is sharp (logits up to ~140), so the logit path uses
split-bf16 (hi+lo) arithmetic giving ~fp32 logits; the post-softmax path is
plain bf16.  G ~= xh xh^T + S + S^T with S = xh xl^T (ll term dropped).

Sharding: data-parallel over batch — one batch element per NeuronCore.
"""
import sys
import numpy as np

for _p in ("/opt/trn_rl_repo", "/root/.axon_site/_ro/trn_rl_repo"):
    if _p not in sys.path:
        sys.path.insert(0, _p)

import ml_dtypes
import concourse.bass as bass
import concourse.tile as tile
import concourse.tile_utils as tile_utils
tile_utils.max_sbuf_usage = 200 * 1024  # cayman: 208KB/partition usable
from concourse import bacc, mybir
from concourse.bass_utils import run_bass_kernel_spmd
from concourse.masks import make_identity

F32 = mybir.dt.float32
BF16 = mybir.dt.bfloat16
FP8 = mybir.dt.float8e4
XL_SCALE = 512.0
AF = mybir.ActivationFunctionType
AX = mybir.AxisListType

C = 512
T = 4096
NCH = C // 128   # 4 partition chunks of the channel dim
NTT = T // 128   # 32 t-tiles (transposed layout)
NTS = T // 512   # 8 t-slices (free-dim tiles)
SCALE = np.float32(64 ** -0.5)

_NC_CACHE = []
_last_in_maps = None


def _emit(nc, tc, ctx, d):
    cs = lambda m: slice(128 * m, 128 * (m + 1))

    persist = ctx.enter_context(tc.tile_pool(name="persist", bufs=1))
    work = ctx.enter_context(tc.tile_pool(name="work", bufs=1))
    outp = ctx.enter_context(tc.tile_pool(name="outp", bufs=4))
    psum = ctx.enter_context(tc.tile_pool(name="psum", bufs=8, space="PSUM"))

    # ---- transposed loads straight from DRAM via X-bar, in T-quarters so
    # the G~ matmul stream can start as soon as the first quarter lands.
    # xT*[p, i, c] holds x^T rows in an arbitrary-but-consistent (p,i)<->t
    # mapping; the Gram contraction is t-permutation invariant.
    xTh = persist.tile([128, NTT, C], BF16, name="xTh", tag="xTh")
    xTl = persist.tile([128, NTT, C], BF16, name="xTl", tag="xTl")
    TQ = 8
    qt, qtiles = T // TQ, NTT // TQ
    for q in range(TQ):
        nc.sync.dma_start(xTh[:, q * qtiles:(q + 1) * qtiles, :],
                          d["xh"][:, q * qt:(q + 1) * qt], transpose=True)
    for q in range(TQ):
        nc.sync.dma_start(xTl[:, q * qtiles:(q + 1) * qtiles, :],
                          d["xl"][:, q * qt:(q + 1) * qt], transpose=True)

    # constants (gpsimd/DVE; ready before PE needs them)
    ident = persist.tile([128, 128], BF16, name="ident", tag="ident")
    make_identity(nc, ident[:])
    one_1 = persist.tile([1, 1], BF16, name="one_1", tag="one_1")
    nc.vector.memset(one_1[:], 1.0)
    t_const = persist.tile([1, 1], BF16, name="tconst", tag="tconst")
    nc.vector.memset(t_const[:], float(T))

    # ---- G~ part 1: Ghh = xh xh^T, i-major so PE chases the transposes ----
    psG = [psum.tile([128, C], F32, name="mm", tag="mm") for _ in range(NCH)]
    for i in range(NTT):
        for m in range(NCH):
            nc.tensor.matmul(psG[m][:], xTh[:, i, cs(m)], xTh[:, i, :],
                             start=(i == 0), stop=False)

    # ---- normal-layout x + weights (issued after the transposes) ----
    x_bf = [persist.tile([128, T], BF16, name=f"xbf{c}", tag=f"xbf{c}")
            for c in range(NCH)]
    for c in range(NCH):
        nc.sync.dma_start(x_bf[c][:], d["xh"][cs(c), :])

    def wchunks(name):
        tiles = []
        for k in range(NCH):
            t_ = persist.tile([128, C], BF16, name=f"{name}{k}", tag=f"{name}{k}")
            nc.sync.dma_start(t_[:], d[name][cs(k), :])
            tiles.append(t_)
        return tiles

    wkt_h, wkt_l = wchunks("wkt_h"), wchunks("wkt_l")
    wqt_h, wqt_l = wchunks("wqt_h"), wchunks("wqt_l")
    wv = wchunks("wv")
    bv = []
    for k in range(NCH):
        t_ = persist.tile([128, 1], BF16, name=f"bv{k}", tag=f"bv{k}")
        nc.sync.dma_start(t_[:], d["bv"][cs(k), :])
        bv.append(t_)
    rows = {}
    for name in ("bq_h", "bq_l", "bk_h", "bk_l"):
        t_ = persist.tile([1, C], BF16, name=name, tag=name)
        nc.sync.dma_start(t_[:], d[name][:])
        rows[name] = t_
    bk_bc = persist.tile([128, C], F32, name="bk_bc", tag="bk_bc")
    nc.sync.dma_start(bk_bc[:], d["bk_bcast"][:])
    bq_c = []
    for k in range(NCH):
        t_ = persist.tile([128, 1], F32, name=f"bqc{k}", tag=f"bqc{k}")
        nc.sync.dma_start(t_[:], d["bq_col"][cs(k), :])
        bq_c.append(t_)

    # ---- G~ part 2: S = xh xl^T in fp8-e4m3 DoubleRow (xl pre-scaled) ----
    xTh8 = persist.tile([128, NTT, C], FP8, name="xTh8", tag="xTh8")
    xTl8 = persist.tile([128, NTT, C], FP8, name="xTl8", tag="xTl8")
    for q in range(TQ):
        qs = slice(q * qtiles, (q + 1) * qtiles)
        nc.vector.tensor_copy(xTh8[:, qs, :], xTh[:, qs, :])
        nc.vector.tensor_scalar_mul(xTl8[:, qs, :], xTl[:, qs, :], XL_SCALE)
    psS = [psum.tile([128, C], F32, name="mm", tag="mm") for _ in range(NCH)]
    for j in range(NTT // 2):
        for m in range(NCH):
            nc.tensor.matmul(psS[m][:], xTh8[:, 2 * j:2 * j + 2, cs(m)],
                             xTl8[:, 2 * j:2 * j + 2, :],
                             start=(j == 0), stop=(j == NTT // 2 - 1),
                             perf_mode=mybir.MatmulPerfMode.DoubleRow)
    S_sb = []
    for m in range(NCH):
        s_sb = work.tile([128, C], BF16, name=f"S{m}", tag=f"S{m}")
        nc.scalar.activation(s_sb[:], psS[m][:], AF.Copy, scale=1.0 / XL_SCALE)
        S_sb.append(s_sb)

    # ---- xs = row sums of x (hi part; lo contribution negligible) ----
    xs_h, xs_l, xsf = [], [], []
    for c in range(NCH):
        f_ = work.tile([128, 1], F32, name=f"xsf{c}", tag=f"xsf{c}")
        nc.vector.reduce_sum(f_[:], x_bf[c][:], axis=AX.X)
        h = work.tile([128, 1], BF16, name=f"xsh{c}", tag=f"xsh{c}")
        l = work.tile([128, 1], BF16, name=f"xsl{c}", tag=f"xsl{c}")
        nc.vector.tensor_copy(h[:], f_[:])
        nc.vector.tensor_sub(l[:], f_[:], h[:])
        xs_h.append(h)
        xs_l.append(l)
        xsf.append(f_)

    # ---- G~ assembly: psG += S + S^T; drain to split-bf16 ----
    Gh, Gl = [], []
    for m in range(NCH):
        nc.tensor.matmul(psG[m][:], ident[:], S_sb[m][:], start=False, stop=False)
        for j in range(NCH):
            nc.tensor.matmul(psG[m][:, cs(j)], S_sb[j][:, cs(m)], ident[:],
                             start=False, stop=(j == NCH - 1))
        gh = work.tile([128, C], BF16, name=f"Gh{m}", tag=f"Gh{m}")
        gl = work.tile([128, C], BF16, name=f"Gl{m}", tag=f"Gl{m}")
        nc.scalar.copy(gh[:], psG[m][:])
        nc.vector.tensor_sub(gl[:], psG[m][:], gh[:])
        Gh.append(gh)
        Gl.append(gl)

    # ---- Z = G~ Wk'^T  (main chunks split-bf16 + rank-1 bias fringe) ----
    Zh, Zl = [], []
    for m in range(NCH):
        ps = psum.tile([128, C], F32, name="mm", tag="mm")
        first = True
        nmm = 3 * NCH
        cnt = 0
        for k in range(NCH):
            for lh, rh in ((Gh[k], wkt_h[k]), (Gh[k], wkt_l[k]), (Gl[k], wkt_h[k])):
                cnt += 1
                nc.tensor.matmul(ps[:], lh[:, cs(m)], rh[:], start=first,
                                 stop=(cnt == nmm))
                first = False
        # fringe xs[e']*bk[d] folded in on DVE, in place on the psum
        nc.vector.scalar_tensor_tensor(ps[:], bk_bc[:], xsf[m][:], ps[:],
                                       op0=mybir.AluOpType.mult,
                                       op1=mybir.AluOpType.add)
        zh = work.tile([128, C], BF16, name=f"Zh{m}", tag=f"Zh{m}")
        zl = work.tile([128, C], BF16, name=f"Zl{m}", tag=f"Zl{m}")
        nc.scalar.copy(zh[:], ps[:])
        nc.vector.tensor_sub(zl[:], ps[:], zh[:])
        Zh.append(zh)
        Zl.append(zl)
    # fringe row of Z:  Zr = xs^T Wk^T + T*bk
    ps = psum.tile([1, C], F32, name="mm", tag="mm")
    first = True
    for k in range(NCH):
        for lh, rh in ((xs_h[k], wkt_h[k]), (xs_h[k], wkt_l[k]), (xs_l[k], wkt_h[k])):
            nc.tensor.matmul(ps[:], lh[:], rh[:], start=first, stop=False)
            first = False
    nc.tensor.matmul(ps[:], t_const[:], rows["bk_h"][:], start=False, stop=False)
    nc.tensor.matmul(ps[:], t_const[:], rows["bk_l"][:], start=False, stop=True)
    Zr_h = work.tile([1, C], BF16, name="Zrh", tag="Zrh")
    Zr_l = work.tile([1, C], BF16, name="Zrl", tag="Zrl")
    nc.scalar.copy(Zr_h[:], ps[:])
    nc.vector.tensor_sub(Zr_l[:], ps[:], Zr_h[:])
    # broadcast Zr (split) to all partitions: ones-column matmul
    ps_bc = psum.tile([128, C], F32, name="mm", tag="mm")
    ones_col = persist.tile([1, 128], BF16, name="ones_col", tag="ones_col")
    nc.vector.memset(ones_col[:], 1.0)
    nc.tensor.matmul(ps_bc[:], ones_col[:], Zr_h[:], start=True, stop=True)
    zrh_bc = work.tile([128, C], BF16, name="zrh_bc", tag="zrh_bc")
    nc.scalar.copy(zrh_bc[:], ps_bc[:])
    ps_bc2 = psum.tile([128, C], F32, name="mm", tag="mm")
    nc.tensor.matmul(ps_bc2[:], ones_col[:], Zr_l[:], start=True, stop=True)
    zrl_bc = work.tile([128, C], BF16, name="zrl_bc", tag="zrl_bc")
    nc.scalar.copy(zrl_bc[:], ps_bc2[:])

    # ---- dots = Wq' Z~ ; fused softmax -> attn_un (bf16) + diag(1/sum) ----
    attn_un, diag = [], []
    for m in range(NCH):
        ps = psum.tile([128, C], F32, name="mm", tag="mm")
        first = True
        nmm = 3 * NCH
        cnt = 0
        for k in range(NCH):
            for lh, rh in ((wqt_h[k], Zh[k]), (wqt_h[k], Zl[k]), (wqt_l[k], Zh[k])):
                cnt += 1
                nc.tensor.matmul(ps[:], lh[:, cs(m)], rh[:], start=first,
                                 stop=(cnt == nmm))
                first = False
        # fringe bq[c]*Zr[d] on DVE, in place on the psum
        nc.vector.scalar_tensor_tensor(ps[:], zrh_bc[:], bq_c[m][:], ps[:],
                                       op0=mybir.AluOpType.mult,
                                       op1=mybir.AluOpType.add)
        nc.vector.scalar_tensor_tensor(ps[:], zrl_bc[:], bq_c[m][:], ps[:],
                                       op0=mybir.AluOpType.mult,
                                       op1=mybir.AluOpType.add)
        nmx = work.tile([128, 1], F32, name=f"nmx{m}", tag=f"nmx{m}")
        nc.vector.reduce_max(nmx[:], ps[:], axis=AX.X, negate=True)
        au = work.tile([128, C], BF16, name=f"au{m}", tag=f"au{m}")
        sm = work.tile([128, 1], F32, name=f"sm{m}", tag=f"sm{m}")
        nc.vector.memset(sm[:], 0.0)
        nc.scalar.activation(au[:], ps[:], AF.Exp, bias=nmx[:], scale=1.0,
                             accum_out=sm[:])
        ri = work.tile([128, 1], F32, name=f"ri{m}", tag=f"ri{m}")
        nc.vector.reciprocal(ri[:], sm[:])
        dg = work.tile([128, 128], BF16, name=f"dg{m}", tag=f"dg{m}")
        nc.vector.tensor_scalar_mul(dg[:], ident[:], ri[:])
        attn_un.append(au)
        diag.append(dg)

    # ---- attn^T (normalized) via matmul with diag(1/sum) rhs ----
    attnT = []
    for j in range(NCH):
        ps = psum.tile([128, C], F32, name="mm", tag="mm")
        for m in range(NCH):
            nc.tensor.matmul(ps[:, cs(m)], attn_un[m][:, cs(j)], diag[m][:],
                             start=True, stop=True)
        at = work.tile([128, C], BF16, name=f"at{j}", tag=f"at{j}")
        nc.scalar.copy(at[:], ps[:])
        attnT.append(at)

    # ---- P~^T = [Wv|bv]^T attn^T ----
    Pt = []
    for jm in range(NCH):
        ps = psum.tile([128, C], F32, name="mm", tag="mm")
        for k in range(NCH):
            nc.tensor.matmul(ps[:], wv[k][:, cs(jm)], attnT[k][:],
                             start=(k == 0), stop=(k == NCH - 1))
        pt = work.tile([128, C], BF16, name=f"pt{jm}", tag=f"pt{jm}")
        nc.scalar.copy(pt[:], ps[:])
        Pt.append(pt)
    # r = attn bv  (as a [1, C] row), then transposed to per-chunk [128, 1]
    ps = psum.tile([1, C], F32, name="mm", tag="mm")
    for k in range(NCH):
        nc.tensor.matmul(ps[:], bv[k][:], attnT[k][:],
                         start=(k == 0), stop=(k == NCH - 1))
    r_b = work.tile([1, C], BF16, name="rb", tag="rb")
    nc.scalar.copy(r_b[:], ps[:])
    rT = []
    ps_rt = psum.tile([128, NCH], F32, name="mm", tag="mm")
    for m in range(NCH):
        nc.tensor.matmul(ps_rt[:, m:m + 1], r_b[:, cs(m)], one_1[:],
                         start=True, stop=True)
    for m in range(NCH):
        rt = work.tile([128, 1], F32, name=f"rT{m}", tag=f"rT{m}")
        nc.vector.tensor_copy(rt[:], ps_rt[:, m:m + 1])
        rT.append(rt)

    # ---- out = P x + r  (bias folded into the activation drain) ----
    for m in range(NCH):
        for ts in range(NTS):
            sl = slice(512 * ts, 512 * (ts + 1))
            ps = psum.tile([128, 512], F32, name="mm", tag="mm")
            for k in range(NCH):
                nc.tensor.matmul(ps[:], Pt[k][:, cs(m)], x_bf[k][:, sl],
                                 start=(k == 0), stop=(k == NCH - 1))
            ob = outp.tile([128, 512], F32, name="ob", tag="ob")
            nc.scalar.activation(ob[:], ps[:], AF.Identity, bias=rT[m][:],
                                 scale=1.0)
            nc.sync.dma_start(d["out"][cs(m), sl], ob[:])


def _declare(nc):
    d = {}
    d["xh"] = nc.declare_dram_parameter("xh", [C, T], BF16, isOutput=False)
    d["xl"] = nc.declare_dram_parameter("xl", [C, T], BF16, isOutput=False)
    for name in ("wqt_h", "wqt_l", "wkt_h", "wkt_l", "wv"):
        d[name] = nc.declare_dram_parameter(name, [C, C], BF16, isOutput=False)
    for name in ("bq_h", "bq_l", "bk_h", "bk_l"):
        d[name] = nc.declare_dram_parameter(name, [1, C], BF16, isOutput=False)
    d["bv"] = nc.declare_dram_parameter("bv", [C, 1], BF16, isOutput=False)
    d["bk_bcast"] = nc.declare_dram_parameter("bk_bcast", [128, C], F32, isOutput=False)
    d["bq_col"] = nc.declare_dram_parameter("bq_col", [C, 1], F32, isOutput=False)
    d["out"] = nc.declare_dram_parameter("out", [C, T], F32, isOutput=True)
    return d


def _build_nc():
    from contextlib import ExitStack
    nc = bacc.Bacc()
    d = _declare(nc)

    with tile.TileContext(nc) as tc:
        with ExitStack() as ctx:
            _emit(nc, tc, ctx, d)
    nc.finalize()
    return nc


def _split_bf16(a):
    h = a.astype(ml_dtypes.bfloat16)
    l = (a.astype(np.float32) - h.astype(np.float32)).astype(ml_dtypes.bfloat16)
    return h, l


def kernel(x, Wq, bq, Wk, bk, Wv, bv):
    x = np.ascontiguousarray(np.asarray(x, dtype=np.float32))
    B = x.shape[0]
    assert x.shape == (B, C, T)

    wqt = np.ascontiguousarray(Wq.T.astype(np.float32) * SCALE)
    wkt = np.ascontiguousarray(Wk.T.astype(np.float32))
    wqt_h, wqt_l = _split_bf16(wqt)
    wkt_h, wkt_l = _split_bf16(wkt)
    bq_h, bq_l = _split_bf16((bq.astype(np.float32) * SCALE)[None, :])
    bk_h, bk_l = _split_bf16(bk.astype(np.float32)[None, :])
    wv_b = Wv.astype(np.float32).astype(ml_dtypes.bfloat16)
    bv_b = bv.astype(np.float32)[:, None].astype(ml_dtypes.bfloat16)

    bk_bcast = np.ascontiguousarray(
        np.broadcast_to(bk.astype(np.float32)[None, :], (128, C)))
    bq_col = np.ascontiguousarray(
        (bq.astype(np.float32) * SCALE)[:, None])
    shared = dict(wqt_h=wqt_h, wqt_l=wqt_l, wkt_h=wkt_h, wkt_l=wkt_l,
                  bq_h=bq_h, bq_l=bq_l, bk_h=bk_h, bk_l=bk_l,
                  wv=wv_b, bv=bv_b, bk_bcast=bk_bcast, bq_col=bq_col)

    in_maps = []
    for b in range(B):
        xh, xl = _split_bf16(x[b])
        in_maps.append(dict(shared, xh=np.ascontiguousarray(xh),
                            xl=np.ascontiguousarray(xl)))

    if not _NC_CACHE:
        _NC_CACHE.append(_build_nc())
    nc = _NC_CACHE[0]

    global _last_in_maps
    _last_in_maps = in_maps

    res = run_bass_kernel_spmd(nc, in_maps, list(range(B)))
    return np.stack([res.results[b]["out"] for b in range(B)], axis=0)
